# revision 43
# baseline (speedup 1.0000x reference)
"""Trainium2 Bass kernel for MixedPerformerAttention (B=2,S=2048,D=2048,H=16).

Sharding: 8 cores = 2 batches x 4 head-slots. Core c (b=c//4, j=c%4) owns
performer heads {2j, 2j+1} (kv head j) and softmax heads {8+2j, 8+2j+1}
(kv head 4+j), plus the matching Wq/Wk/Wv rows and Wo columns. Each core
computes a [S, D] partial output projection; the host sums 4 partials/batch.

Two-pass structure keeps the tensor engine continuously busy (PE ramps to
max clock only after ~3us of uninterrupted work):
  pass 1: q/k/v projections + rotary + performer FAVOR+ features
          (pq/pk in both layouts) + per-chunk kv outer products + prefix sums.
  pass 2: softmax attention (scores/exp/AV/denominator), performer causal
          linear attention (all chunk matmuls dependency-free thanks to the
          precomputed exclusive-prefix kv tensors), output projection.

dtypes: fp32r for every matmul with free-size >= 256 (full PE rate there),
bf16 only in the performer branch (free=128 matmuls where fp32r is 4x slower)
and for Wo/attn in the output projection. The exact reference stabilizers
(per-token q max, host-shipped global k max, sq, 1/sqrt(M)) are reproduced
so num/(den+EPS) matches the reference's EPS=1e-6 guard.
"""

import sys

sys.path.insert(0, "/opt/trn_rl_repo")

import numpy as np

import concourse.bass as bass
import concourse.tile as tile
from concourse import bacc, bass_isa, mybir
from concourse._compat import with_exitstack

F32 = mybir.dt.float32
F32R = mybir.dt.float32r
BF16 = mybir.dt.bfloat16
AF = mybir.ActivationFunctionType
AX = mybir.AxisListType
ALU = mybir.AluOpType

B, S, D = 2, 2048, 2048
H, KVH, HD = 16, 8, 128
NPH, M, C = 8, 128, 128
SCALE = HD ** -0.5
EPS = 1e-6
LNM = float(np.log(np.sqrt(M)))
HDQ = HD ** -0.25

NJ, JW, NB, ND = 4, 512, 16, 16


def _r(ap):
    return ap.bitcast(F32R)


@with_exitstack
def _emit(ctx, tc, aps, debug=False):
    nc = tc.nc
    hsT, wq, wk, wv, wo = aps["hsT"], aps["wq"], aps["wk"], aps["wv"], aps["wo"]
    out = aps["out"]

    pers = ctx.enter_context(tc.tile_pool(name="pers", bufs=1))

    # ---------------- persistent tiles ----------------
    omgx = pers.tile([128, 128], BF16, name="omgx", tag="omgx")
    identb = pers.tile([128, 128], BF16, name="identb", tag="identb")
    triu = pers.tile([128, 128], BF16, name="triu", tag="triu")
    cbt = pers.tile([128, 2], BF16, name="cbt", tag="cbt")  # col0 ones, col1 .5*HD^-.5
    onesc = pers.tile([128, 1], F32R, name="onesc", tag="onesc")
    stkcol = pers.tile([128, 1], F32, name="stkcol", tag="stkcol")  # -stabk - LNM
    epsc = pers.tile([128, 1], F32, name="epsc", tag="epsc")
    dmask = [pers.tile([128, 512], BF16, name=f"dmask{t}", tag=f"dmask{t}")
             for t in range(4)]
    wo_t = [pers.tile([128, 2048], BF16, name=f"wo{i}", tag=f"wo{i}") for i in range(4)]

    qts = [pers.tile([128, 2048], F32R, name=f"qts{h}", tag=f"qts{h}") for h in range(2)]
    kts = pers.tile([128, 2048], F32R, name="kts", tag="kts")
    vs_t = [pers.tile([128, 128], F32R, name=f"vs{i}", tag=f"vs{i}") for i in range(NB)]
    vaug = [pers.tile([128, 129], BF16, name=f"vaug{i}", tag=f"vaug{i}") for i in range(NB)]
    pqT = [[pers.tile([128, 128], BF16, name=f"pqT{i}_{h}", tag=f"pqT{i}_{h}")
            for h in range(2)] for i in range(NB)]
    pkT = [pers.tile([128, 128], BF16, name=f"pkT{i}", tag=f"pkT{i}") for i in range(NB)]
    kvb = [pers.tile([128, 129], BF16, name=f"kvb{i}", tag=f"kvb{i}") for i in range(1, NB)]
    kvf = pers.tile([128, 129], F32, name="kvf", tag="kvf")

    # ---------------- pass 1 ----------------
    with tc.tile_pool(name="w1", bufs=1) as w1, \
         tc.tile_pool(name="hstp", bufs=22) as hst_p, \
         tc.tile_pool(name="rot", bufs=2) as rot_p, \
         tc.tile_pool(name="sm1", bufs=3) as sm1, \
         tc.tile_pool(name="ps1", bufs=1, space="PSUM") as ps1:

        wq_t = [w1.tile([128, 512], F32R, name=f"wq{d}", tag=f"wq{d}") for d in range(ND)]
        wk_t = [w1.tile([128, 256], F32R, name=f"wk{d}", tag=f"wk{d}") for d in range(ND)]
        wv_t = [w1.tile([128, 256], F32R, name=f"wv{d}", tag=f"wv{d}") for d in range(ND)]

        # compute-critical DMAs first: J0 activations interleaved with q weights
        hst0 = [hst_p.tile([128, JW], F32R, name="hst", tag="hst") for _ in range(ND)]
        for d in range(ND):
            nc.sync.dma_start(hst0[d][:], hsT[d * 128:(d + 1) * 128, 0:JW])
            nc.sync.dma_start(wq_t[d][:], wq[d * 128:(d + 1) * 128, :])
        co0 = rot_p.tile([128, JW], F32, name="cos", tag="cos")
        si0 = rot_p.tile([128, JW], F32, name="sin", tag="sin")
        nc.sync.dma_start(co0[:], aps["cost"][:, 0:JW])
        nc.sync.dma_start(si0[:], aps["sintn"][:, 0:JW])
        for d in range(ND):
            nc.sync.dma_start(wk_t[d][:], wk[d * 128:(d + 1) * 128, :])
            nc.sync.dma_start(wv_t[d][:], wv[d * 128:(d + 1) * 128, :])
        nc.sync.dma_start(omgx[:], aps["omgx"][:])
        nc.sync.dma_start(identb[:], aps["identb"][:])
        nc.sync.dma_start(triu[:], aps["triu"][:])
        nc.sync.dma_start(cbt[:], aps["cbt"][:])
        nc.sync.dma_start(onesc[:], aps["onesc"][:])
        nc.sync.dma_start(stkcol[:], aps["stkcol"][:])
        nc.sync.dma_start(epsc[:], aps["epsc"][:])
        for i in range(NB):
            nc.sync.dma_start(vaug[i][:, 128:129], aps["onesbc"][:])
        nc.vector.memset(kvf[:].bitcast(mybir.dt.uint32), 0)

        # pre-create all J's hst tiles so each J+1's first tiles (whose pool
        # buffers are free early) can be DMA'd ahead of the J boundary
        hst_all = [hst0] + [[hst_p.tile([128, JW], F32R, name="hst", tag="hst")
                             for _ in range(ND)] for _ in range(NJ - 1)]
        for d in range(6):
            nc.sync.dma_start(hst_all[1][d][:],
                              hsT[d * 128:(d + 1) * 128, JW:2 * JW])

        ones_b, hcol = cbt[:, 0:1], cbt[:, 1:2]

        def rotary(ps, dst):
            swp = rot_p.tile([128, JW], F32, name="rswp", tag="rswp", bufs=2)
            nc.vector.tensor_copy(swp[0:64, :], ps[64:128, :])
            nc.vector.tensor_copy(swp[64:128, :], ps[0:64, :])
            tmp = rot_p.tile([128, JW], F32, name="rtmp", tag="rtmp", bufs=2)
            nc.vector.tensor_mul(tmp[:], swp[:], si[:])
            nc.vector.tensor_mul(swp[:], ps[:], co[:])
            nc.vector.tensor_add(dst, swp[:], tmp[:])

        def emit_trans(Jp, pk_toks, pq_toks):
            # transposes + kv outer products for block Jp (exps long done)
            for t in range(4):
                c = 4 * Jp + t
                trk = ps1.tile([128, 128], BF16, name="trk", tag="work", bufs=4)
                nc.tensor.transpose(trk[:], pk_toks[t][:], identb[:])
                nc.vector.tensor_copy(pkT[c][:], trk[:])
                for h in range(2):
                    trq = ps1.tile([128, 128], BF16, name="trq", tag="work", bufs=4)
                    nc.tensor.transpose(trq[:], pq_toks[t][h][:], identb[:])
                    nc.vector.tensor_copy(pqT[c][h][:], trq[:])
                kvc = ps1.tile([128, 129], F32, name="kvc", tag="work", bufs=4)
                nc.tensor.matmul(kvc[:], pk_toks[t][:], vaug[c][:],
                                 start=True, stop=True)
                if c > 0:
                    nc.vector.tensor_copy(kvb[c - 1][:], kvf[:])
                nc.vector.tensor_add(kvf[:], kvf[:], kvc[:])

        prev_f = None
        for J in range(NJ):
            s0 = J * JW
            if J == 0:
                hst, co, si = hst0, co0, si0
            else:
                hst = hst_all[J]
                for d in range(6, ND):
                    nc.sync.dma_start(hst[d][:], hsT[d * 128:(d + 1) * 128, s0:s0 + JW])
                if J + 1 < NJ:
                    for d in range(6):
                        nc.sync.dma_start(hst_all[J + 1][d][:],
                                          hsT[d * 128:(d + 1) * 128,
                                              s0 + JW:s0 + 2 * JW])
                co = rot_p.tile([128, JW], F32, name="cos", tag="cos")
                si = rot_p.tile([128, JW], F32, name="sin", tag="sin")
                nc.sync.dma_start(co[:], aps["cost"][:, s0:s0 + JW])
                nc.sync.dma_start(si[:], aps["sintn"][:, s0:s0 + JW])

            # --- projections ---
            qtp = [sm1.tile([128, JW], BF16, name=f"qtp{g}", tag=f"qtp{g}", bufs=1)
                   for g in range(2)]
            for g in range(4):
                ps = ps1.tile([128, JW], F32, name="pp", tag="pp", bufs=2)
                for d in range(ND):
                    nc.tensor.matmul(ps[:], wq_t[d][:, g * 128:(g + 1) * 128],
                                     hst[d][:], start=(d == 0), stop=(d == ND - 1))
                if g < 2:
                    rotary(ps, qtp[g][:])
                else:
                    rotary(ps, qts[g - 2][:, s0:s0 + JW])
            # J-1's transposes/kv: PE work whose scalar-exp inputs completed
            # during this block's q matmuls
            if prev_f is not None:
                emit_trans(J - 1, *prev_f)
            ktp = sm1.tile([128, JW], BF16, name="ktp", tag="ktp", bufs=1)
            for g in range(2):
                ps = ps1.tile([128, JW], F32, name="pp", tag="pp", bufs=2)
                for d in range(ND):
                    nc.tensor.matmul(ps[:], wk_t[d][:, g * 128:(g + 1) * 128],
                                     hst[d][:], start=(d == 0), stop=(d == ND - 1))
                if g == 0:
                    rotary(ps, ktp[:])
                else:
                    rotary(ps, kts[:, s0:s0 + JW])
            # --- v projections + performer features, interleaved so the PE
            # always has independent matmul work while the vector/scalar
            # engines run the rotary and exp chains.
            def v_proj(sb):
                blk = J * 4 + sb
                ps = ps1.tile([128, 256], F32, name="ppv", tag="pp", bufs=2)
                for d in range(ND):
                    nc.tensor.matmul(ps[:], hst[d][:, sb * 128:(sb + 1) * 128],
                                     wv_t[d][:], start=(d == 0), stop=(d == ND - 1))
                nc.vector.tensor_copy(vaug[blk][:, 0:128], ps[:, 0:128])
                nc.scalar.copy(vs_t[blk][:], ps[:, 128:256])

            v_proj(0)
            v_proj(1)
            v_proj(2)
            v_proj(3)
            # q features (need only rotQ of heads 0/1 — done during k/v work)
            pq_toks = []
            for t in range(4):
                hp = []
                for h in range(2):
                    cs = t * 128
                    q2 = sm1.tile([128, 128], BF16, name="q2", tag="q2", bufs=2)
                    nc.vector.tensor_mul(q2[:], qtp[h][:, cs:cs + 128],
                                         qtp[h][:, cs:cs + 128])
                    fq = ps1.tile([128, 129], F32, name="fq", tag="work", bufs=4)
                    nc.tensor.matmul(fq[:, 0:128], qtp[h][:, cs:cs + 128], omgx[:],
                                     start=True, stop=True)
                    nc.tensor.matmul(fq[:, 128:129], q2[:], hcol, start=True, stop=True)
                    mx = sm1.tile([128, 1], F32, name="mx", tag="mx", bufs=4)
                    nc.vector.reduce_max(mx[:], fq[:, 0:128], axis=AX.X)
                    nc.vector.tensor_add(mx[:], mx[:], fq[:, 128:129])
                    nc.vector.tensor_scalar(mx[:], mx[:], -1.0, -LNM,
                                            ALU.mult, ALU.add)
                    pq_tok = sm1.tile([128, 128], BF16, name="pq", tag="pq", bufs=10)
                    nc.scalar.activation(pq_tok[:], fq[:, 0:128], AF.Exp,
                                         bias=mx[:], scale=1.0)
                    hp.append(pq_tok)
                pq_toks.append(hp)
            # k features (need rotK — done during the v work above)
            pk_toks = []
            for t in range(4):
                cs = t * 128
                k2 = sm1.tile([128, 128], BF16, name="k2", tag="k2", bufs=2)
                nc.vector.tensor_mul(k2[:], ktp[:, cs:cs + 128], ktp[:, cs:cs + 128])
                fk = ps1.tile([128, 129], F32, name="fk", tag="work", bufs=4)
                nc.tensor.matmul(fk[:, 0:128], ktp[:, cs:cs + 128], omgx[:],
                                 start=True, stop=True)
                nc.tensor.matmul(fk[:, 128:129], k2[:], hcol, start=True, stop=True)
                bk = sm1.tile([128, 1], F32, name="bk", tag="bk", bufs=4)
                nc.vector.tensor_scalar(bk[:], fk[:, 128:129], -1.0, stkcol[:],
                                        ALU.mult, ALU.add)
                pk_tok = sm1.tile([128, 128], BF16, name="pk", tag="pk", bufs=6)
                nc.scalar.activation(pk_tok[:], fk[:, 0:128], AF.Exp,
                                     bias=bk[:], scale=1.0)
                pk_toks.append(pk_tok)
            prev_f = (pk_toks, pq_toks)
            if J == 1:
                # pass-2-only tensors: issued late so they never delay the
                # compute-critical activation stream
                for t in range(4):
                    nc.sync.dma_start(dmask[t][:],
                                      aps["masks"][t * 128:(t + 1) * 128, :])
                for i in range(4):
                    nc.sync.dma_start(wo_t[i][:], wo[i * 128:(i + 1) * 128, :])
        emit_trans(NJ - 1, *prev_f)

    # ---------------- pass 2 ----------------
    with tc.tile_pool(name="pt2", bufs=3) as pt_p, \
         tc.tile_pool(name="sm2", bufs=3) as sm2, \
         tc.tile_pool(name="at2", bufs=2) as at_p, \
         tc.tile_pool(name="ost2", bufs=3) as ost_p, \
         tc.tile_pool(name="ps2", bufs=1, space="PSUM") as ps2:

        def emit_outproj(Jp, atiles):
            s0p = Jp * JW
            for sb in range(4):
                ostb = ost_p.tile([128, D], F32, name="ostb", tag="ost", bufs=3)
                for oc in range(4):
                    pso = ps2.tile([128, JW], F32, name="pso", tag="po", bufs=2)
                    for i in range(4):
                        nc.tensor.matmul(pso[:],
                                         atiles[i][:, sb * 128:(sb + 1) * 128],
                                         wo_t[i][:, oc * 512:(oc + 1) * 512],
                                         start=(i == 0), stop=(i == 3))
                    if oc % 2 == 0:
                        nc.vector.tensor_copy(ostb[:, oc * 512:(oc + 1) * 512],
                                              pso[:])
                    else:
                        nc.scalar.copy(ostb[:, oc * 512:(oc + 1) * 512], pso[:])
                nc.sync.dma_start(
                    out[s0p + sb * 128: s0p + (sb + 1) * 128, :], ostb[:])

        prev_at = None
        for J in range(NJ):
            s0 = J * JW
            nblk = 4 * J + 4
            # --- softmax heads ---
            av2 = [ps2.tile([128, JW], F32, name=f"av{h}", tag=f"av{h}", bufs=1)
                   for h in range(2)]
            dnacc = [sm2.tile([1, JW], F32, name=f"dna{h}", tag=f"dna{h}", bufs=1)
                     for h in range(2)]
            # software-pipelined: issue st(i) for both heads, then consume
            # pt(i-1) — the scalar exp always has a full block-time to finish
            # before the PE needs its output. Diagonal blocks (i >= 4J) only
            # compute the causally-needed column range [t*128, 512).
            pts = [None, None]
            pcs = 0
            for i in range(nblk + 1):
                npt = [None, None]
                ncs = (i - 4 * J) * 128 if i >= 4 * J else 0
                if i < nblk:
                    for h in range(2):
                        st = ps2.tile([128, JW], F32, name="st", tag="pp", bufs=2)
                        nc.tensor.matmul(st[:, ncs:JW], kts[:, i * 128:(i + 1) * 128],
                                         qts[h][:, s0 + ncs:s0 + JW],
                                         start=True, stop=True)
                        pt = pt_p.tile([128, JW], F32R, name="pt", tag="pt", bufs=4)
                        nc.scalar.activation(pt[:, ncs:JW], st[:, ncs:JW], AF.Exp,
                                             bias=0.0, scale=SCALE)
                        if i >= 4 * J:
                            nc.vector.tensor_mul(pt[:, ncs:JW], pt[:, ncs:JW],
                                                 dmask[i - 4 * J][:, ncs:JW])
                        npt[h] = pt
                if i > 0:
                    for h in range(2):
                        nc.tensor.matmul(av2[h][:, pcs:JW], vs_t[i - 1][:],
                                         pts[h][:, pcs:JW],
                                         start=(i == 1), stop=(i == nblk))
                        dnf = sm2.tile([128, JW], F32, name="dnf", tag="dnf",
                                       bufs=4)
                        nc.gpsimd.partition_all_reduce(
                            dnf[:, pcs:JW], pts[h][:, pcs:JW], 128,
                            bass_isa.ReduceOp.add)
                        if i == 1:
                            nc.gpsimd.tensor_copy(dnacc[h][:], dnf[0:1, :])
                        else:
                            nc.gpsimd.tensor_add(dnacc[h][:, pcs:JW],
                                                 dnacc[h][:, pcs:JW],
                                                 dnf[0:1, pcs:JW])
                pts = npt
                pcs = ncs
            at_s = [at_p.tile([128, JW], BF16, name=f"ats{h}", tag=f"ats{h}")
                    for h in range(2)]
            for h in range(2):
                dnr = sm2.tile([1, JW], F32, name="dnr", tag="dnr", bufs=2)
                nc.scalar.activation(dnr[:], dnacc[h][:], AF.Ln, bias=0.0, scale=1.0)
                nc.scalar.activation(dnr[:], dnr[:], AF.Exp, bias=0.0, scale=-1.0)
                bcs = sm2.tile([128, JW], F32, name="bcs", tag="bcs", bufs=2)
                nc.gpsimd.partition_broadcast(bcs[:], dnr[:])
                nc.vector.tensor_mul(at_s[h][:], av2[h][:], bcs[:])

            # --- performer heads: out = num/(den+EPS), den batched per (J,h)
            # into one [1,512] PSUM row (reusing the dn tag) so the
            # reciprocal/broadcast chain runs once per head per block.
            at_pf = [at_p.tile([128, JW], BF16, name=f"atp{h}", tag=f"atp{h}")
                     for h in range(2)]
            denJ = [ps2.tile([1, JW], F32, name=f"denp{h}", tag=f"dn{h}", bufs=1)
                    for h in range(2)]
            numJ = [sm2.tile([128, JW], F32, name=f"numc{h}", tag="numc", bufs=2)
                    for h in range(2)]
            for t in range(4):
                c = 4 * J + t
                cs = t * 128
                # both heads' aT first, so aM (vector) is ready by the time
                # the intra matmuls need it; inter matmuls need only kvb.
                aMs = [None, None]
                for h in range(2):
                    aT = ps2.tile([128, 128], F32, name="aT", tag="pp", bufs=2)
                    nc.tensor.matmul(aT[:], pkT[c][:], pqT[c][h][:],
                                     start=True, stop=True)
                    aM = sm2.tile([128, 128], BF16, name="aM", tag="aM", bufs=4)
                    nc.vector.tensor_mul(aM[:], aT[:], triu[:])
                    aMs[h] = aM
                for h in range(2):
                    num = ps2.tile([128, 128], F32, name="num", tag="pp", bufs=2)
                    if c > 0:
                        nc.tensor.matmul(num[:], kvb[c - 1][:, 0:128], pqT[c][h][:],
                                         start=True, stop=False)
                        nc.tensor.matmul(denJ[h][:, cs:cs + 128],
                                         kvb[c - 1][:, 128:129], pqT[c][h][:],
                                         start=True, stop=False)
                    nc.tensor.matmul(num[:], vaug[c][:, 0:128], aMs[h][:],
                                     start=(c == 0), stop=True)
                    nc.tensor.matmul(denJ[h][:, cs:cs + 128],
                                     vaug[c][:, 128:129], aMs[h][:],
                                     start=(c == 0), stop=True)
                    nc.vector.tensor_copy(numJ[h][:, cs:cs + 128], num[:])
            for h in range(2):
                rcp = sm2.tile([1, JW], F32, name="rcp", tag="rcp", bufs=2)
                nc.scalar.activation(rcp[:], denJ[h][:], AF.Ln,
                                     bias=epsc[0:1, :], scale=1.0)
                nc.scalar.activation(rcp[:], rcp[:], AF.Exp, bias=0.0, scale=-1.0)
                bcp = sm2.tile([128, JW], F32, name="bcp", tag="bcp", bufs=2)
                nc.gpsimd.partition_broadcast(bcp[:], rcp[:])
                nc.vector.tensor_mul(at_pf[h][:], numJ[h][:], bcp[:])

            # --- output projection of the PREVIOUS block: fills the PE while
            # this block's at_s/at_pf vector chains complete ---
            if prev_at is not None:
                emit_outproj(J - 1, prev_at)
            prev_at = [at_pf[0], at_pf[1], at_s[0], at_s[1]]

            if debug:
                for h in range(2):
                    nc.sync.dma_start(aps["dbg_ats"][h * 128:(h + 1) * 128, s0:s0 + JW],
                                      at_s[h][:])
                    nc.sync.dma_start(aps["dbg_atp"][h * 128:(h + 1) * 128, s0:s0 + JW],
                                      at_pf[h][:])
        emit_outproj(NJ - 1, prev_at)
        if debug:
            nc.sync.dma_start(aps["dbg_kts"][:], kts[:].bitcast(F32))
            for h in range(2):
                nc.sync.dma_start(aps["dbg_qts"][h * 128:(h + 1) * 128, :],
                                  qts[h][:].bitcast(F32))
            for c in range(NB):
                nc.sync.dma_start(aps["dbg_pk"][:, c * 128:(c + 1) * 128], pkT[c][:])
                for h in range(2):
                    nc.sync.dma_start(aps["dbg_pq"][h * 128:(h + 1) * 128,
                                                    c * 128:(c + 1) * 128], pqT[c][h][:])


def _pin_act_tables():
    """Make every ACT table-set except natural_log_exp_and_others ineligible so
    the loader never thrashes between table sets. Set ids are positional, so
    keep the dict size/order and just empty the others."""
    import concourse.bacc as bacc_mod
    if getattr(bacc_mod, "_act_tables_pinned", False):
        return
    orig = bacc_mod.get_activation_tables

    def patched(arch):
        t = orig(arch)
        return {k: (v if k == "natural_log_exp_and_others" else set())
                for k, v in t.items()}

    bacc_mod.get_activation_tables = patched
    bacc_mod._act_tables_pinned = True


def build(debug=False):
    _pin_act_tables()
    nc = bacc.Bacc("TRN2", target_bir_lowering=False, debug=False, num_devices=8)
    shapes = {
        "hsT": [D, S], "wq": [D, 512], "wk": [D, 256], "wv": [D, 256],
        "wo": [512, D], "cost": [128, S], "sintn": [128, S],
        "omgx": [128, 128], "identb": [128, 128], "triu": [128, 128],
        "cbt": [128, 2], "onesc": [128, 1], "stkcol": [128, 1],
        "epsc": [128, 1],
        "masks": [512, 512], "onesbc": [128, 1],
    }
    BF16_INS = {"omgx", "identb", "triu", "cbt", "masks", "onesbc"}
    F32R_INS = {"hsT", "wq", "wk", "wv", "onesc"}

    def _dt(n):
        if n == "wo":
            return BF16
        if n in BF16_INS:
            return BF16
        return F32R if n in F32R_INS else F32
    aps = {n: nc.dram_tensor(n, s, _dt(n), kind="ExternalInput").ap()
           for n, s in shapes.items()}
    aps["out"] = nc.dram_tensor("out", [S, D], F32, kind="ExternalOutput").ap()
    if debug:
        for n, s, dt in [("dbg_qts", [256, S], F32), ("dbg_kts", [128, S], F32),
                         ("dbg_ats", [256, S], BF16), ("dbg_atp", [256, S], BF16),
                         ("dbg_pq", [256, S], BF16), ("dbg_pk", [128, S], BF16)]:
            aps[n] = nc.dram_tensor(n, s, dt, kind="ExternalOutput").ap()
    with tile.TileContext(nc) as tc:
        _emit(tc, aps, debug=debug)
    nc.compile()
    return nc


def host_prep(hidden_states, cos, sin, Wq, Wk, Wv, Wo, omega):
    """Slice/transpose full inputs into 8 per-core input maps."""
    import ml_dtypes
    f32 = np.float32
    bf16 = ml_dtypes.bfloat16
    hs = np.asarray(hidden_states, f32)
    cos = np.asarray(cos, f32)
    sin = np.asarray(sin, f32)
    Wq, Wk, Wv, Wo = (np.asarray(x, f32) for x in (Wq, Wk, Wv, Wo))
    omega = np.asarray(omega, f32)

    omgx = np.ascontiguousarray((omega * HDQ).T).astype(bf16)
    identb = np.eye(128, dtype=f32).astype(bf16)
    triu = np.triu(np.ones((128, 128), f32)).astype(bf16)  # aT layout [k,q]: keep k<=q
    cbt = np.zeros((128, 2), f32)
    cbt[:, 0] = 1.0
    cbt[:, 1] = 0.5 * HD ** -0.5
    cbt = cbt.astype(bf16)
    onesc = np.ones((128, 1), f32)
    onesbc = np.ones((128, 1), f32).astype(bf16)
    masks = np.zeros((512, 512), f32)  # diag-block masks, 4x128
    pidx = np.arange(128)[:, None]
    cidx = np.arange(512)[None, :]
    for t in range(4):
        masks[t * 128:(t + 1) * 128, :] = (cidx >= t * 128 + pidx)
    masks = masks.astype(bf16)

    # stabk per (b, perf kv head j): max over (s,m) of proj_k (pre-stab)
    stab = np.zeros((B, 4), f32)
    kproj = np.einsum("bsd,od->bso", hs, Wk[0:512]).reshape(B, S, 4, HD)
    khalf = np.concatenate([-kproj[..., 64:], kproj[..., :64]], axis=-1)
    krot = kproj * cos[:, :, None, :] + khalf * sin[:, :, None, :]
    for b in range(B):
        for j in range(4):
            pj = (krot[b, :, j] * HDQ) @ omega.T
            stab[b, j] = pj.max()

    in_maps = []
    for core in range(8):
        b, j = divmod(core, 4)
        heads = [2 * j, 2 * j + 1, 8 + 2 * j, 8 + 2 * j + 1]
        qrows = np.concatenate([Wq[h * 128:(h + 1) * 128] for h in heads])
        kvh = [j, 4 + j]
        krows = np.concatenate([Wk[g * 128:(g + 1) * 128] for g in kvh])
        vrows = np.concatenate([Wv[g * 128:(g + 1) * 128] for g in kvh])
        wocols = np.concatenate([Wo[:, h * 128:(h + 1) * 128] for h in heads],
                                axis=1)
        sh = sin[b, :, 0:64]
        sintn = np.ascontiguousarray(np.concatenate([-sh, sh], axis=1).T)
        stkcol = np.full((128, 1), -stab[b, j] - LNM, f32)
        in_maps.append({
            "hsT": np.ascontiguousarray(hs[b].T),
            "wq": np.ascontiguousarray(qrows.T),
            "wk": np.ascontiguousarray(krows.T),
            "wv": np.ascontiguousarray(vrows.T),
            "wo": np.ascontiguousarray(wocols.T).astype(bf16),
            "cost": np.ascontiguousarray(cos[b].T),
            "sintn": sintn,
            "omgx": omgx, "identb": identb, "triu": triu,
            "cbt": cbt, "onesc": onesc, "stkcol": stkcol,
            "epsc": np.full((128, 1), EPS, f32),
            "masks": masks, "onesbc": onesbc,
        })
    return in_maps


_NC_CACHE = {}


def kernel(**inputs):
    from concourse.bass_utils import run_bass_kernel_spmd
    if "nc" not in _NC_CACHE:
        _NC_CACHE["nc"] = build(debug=False)
    nc = _NC_CACHE["nc"]
    in_maps = host_prep(**inputs)
    res = run_bass_kernel_spmd(nc, in_maps, core_ids=list(range(8)))
    out = np.zeros((B, S, D), np.float32)
    for core in range(8):
        out[core // 4] += res.results[core]["out"]
    return out


# revision 44
# speedup vs baseline: 3.1044x; 3.1044x over previous
"""Trainium2 Bass kernel for MixedPerformerAttention (B=2,S=2048,D=2048,H=16).

Sharding: 8 cores = 2 batches x 4 head-slots. Core c (b=c//4, j=c%4) owns
performer heads {2j, 2j+1} (kv head j) and softmax heads {8+2j, 8+2j+1}
(kv head 4+j), plus the matching Wq/Wk/Wv rows and Wo columns. Each core
computes a [S, D] partial output projection; the host sums 4 partials/batch.

Two-pass structure keeps the tensor engine continuously busy (PE ramps to
max clock only after ~3us of uninterrupted work):
  pass 1: q/k/v projections + rotary + performer FAVOR+ features
          (pq/pk in both layouts) + per-chunk kv outer products + prefix sums.
  pass 2: softmax attention (scores/exp/AV/denominator), performer causal
          linear attention (all chunk matmuls dependency-free thanks to the
          precomputed exclusive-prefix kv tensors), output projection.

dtypes: fp32r for every matmul with free-size >= 256 (full PE rate there),
bf16 only in the performer branch (free=128 matmuls where fp32r is 4x slower)
and for Wo/attn in the output projection. The exact reference stabilizers
(per-token q max, host-shipped global k max, sq, 1/sqrt(M)) are reproduced
so num/(den+EPS) matches the reference's EPS=1e-6 guard.
"""

import sys

sys.path.insert(0, "/opt/trn_rl_repo")

import numpy as np

import concourse.bass as bass
import concourse.tile as tile
from concourse import bacc, bass_isa, mybir
from concourse._compat import with_exitstack

F32 = mybir.dt.float32
F32R = mybir.dt.float32r
BF16 = mybir.dt.bfloat16
AF = mybir.ActivationFunctionType
AX = mybir.AxisListType
ALU = mybir.AluOpType

B, S, D = 2, 2048, 2048
H, KVH, HD = 16, 8, 128
NPH, M, C = 8, 128, 128
SCALE = HD ** -0.5
EPS = 1e-6
LNM = float(np.log(np.sqrt(M)))
HDQ = HD ** -0.25

NJ, JW, NB, ND = 4, 512, 16, 16


def _r(ap):
    return ap.bitcast(F32R)


@with_exitstack
def _emit(ctx, tc, aps, debug=False):
    nc = tc.nc
    hsT, wq, wk, wv, wo = aps["hsT"], aps["wq"], aps["wk"], aps["wv"], aps["wo"]
    out = aps["out"]

    pers = ctx.enter_context(tc.tile_pool(name="pers", bufs=1))

    # ---------------- persistent tiles ----------------
    omgx = pers.tile([128, 128], BF16, name="omgx", tag="omgx")
    identb = pers.tile([128, 128], BF16, name="identb", tag="identb")
    triu = pers.tile([128, 128], BF16, name="triu", tag="triu")
    cbt = pers.tile([128, 2], BF16, name="cbt", tag="cbt")  # col0 ones, col1 .5*HD^-.5
    onesc = pers.tile([128, 1], F32R, name="onesc", tag="onesc")
    stkcol = pers.tile([128, 1], F32, name="stkcol", tag="stkcol")  # -stabk - LNM
    epsc = pers.tile([128, 1], F32, name="epsc", tag="epsc")
    dmask = [pers.tile([128, 512], BF16, name=f"dmask{t}", tag=f"dmask{t}")
             for t in range(4)]
    wo_t = [pers.tile([128, 2048], BF16, name=f"wo{i}", tag=f"wo{i}") for i in range(4)]

    qts = [pers.tile([128, 2048], F32R, name=f"qts{h}", tag=f"qts{h}") for h in range(2)]
    kts = pers.tile([128, 2048], F32R, name="kts", tag="kts")
    vs_t = [pers.tile([128, 128], F32R, name=f"vs{i}", tag=f"vs{i}") for i in range(NB)]
    vaug = [pers.tile([128, 129], BF16, name=f"vaug{i}", tag=f"vaug{i}") for i in range(NB)]
    pqT = [[pers.tile([128, 128], BF16, name=f"pqT{i}_{h}", tag=f"pqT{i}_{h}")
            for h in range(2)] for i in range(NB)]
    pkT = [pers.tile([128, 128], BF16, name=f"pkT{i}", tag=f"pkT{i}") for i in range(NB)]
    kvb = [pers.tile([128, 129], BF16, name=f"kvb{i}", tag=f"kvb{i}") for i in range(1, NB)]
    kvf = pers.tile([128, 129], F32, name="kvf", tag="kvf")

    # ---------------- pass 1 ----------------
    with tc.tile_pool(name="w1", bufs=1) as w1, \
         tc.tile_pool(name="hstp", bufs=22) as hst_p, \
         tc.tile_pool(name="rot", bufs=2) as rot_p, \
         tc.tile_pool(name="sm1", bufs=3) as sm1, \
         tc.tile_pool(name="ps1", bufs=1, space="PSUM") as ps1:

        wq_t = [w1.tile([128, 512], F32R, name=f"wq{d}", tag=f"wq{d}") for d in range(ND)]
        wk_t = [w1.tile([128, 256], F32R, name=f"wk{d}", tag=f"wk{d}") for d in range(ND)]
        wv_t = [w1.tile([128, 256], F32R, name=f"wv{d}", tag=f"wv{d}") for d in range(ND)]

        # compute-critical DMAs first: J0 activations interleaved with q weights
        hst0 = [hst_p.tile([128, JW], F32R, name="hst", tag="hst") for _ in range(ND)]
        for d in range(ND):
            nc.sync.dma_start(hst0[d][:], hsT[d * 128:(d + 1) * 128, 0:JW])
            nc.sync.dma_start(wq_t[d][:], wq[d * 128:(d + 1) * 128, :])
        co0 = rot_p.tile([128, JW], F32, name="cos", tag="cos")
        si0 = rot_p.tile([128, JW], F32, name="sin", tag="sin")
        nc.sync.dma_start(co0[:], aps["cost"][:, 0:JW])
        nc.sync.dma_start(si0[:], aps["sintn"][:, 0:JW])
        for d in range(ND):
            nc.sync.dma_start(wk_t[d][:], wk[d * 128:(d + 1) * 128, :])
            nc.sync.dma_start(wv_t[d][:], wv[d * 128:(d + 1) * 128, :])
        nc.sync.dma_start(omgx[:], aps["omgx"][:])
        nc.sync.dma_start(identb[:], aps["identb"][:])
        nc.sync.dma_start(triu[:], aps["triu"][:])
        nc.sync.dma_start(cbt[:], aps["cbt"][:])
        nc.sync.dma_start(onesc[:], aps["onesc"][:])
        nc.sync.dma_start(stkcol[:], aps["stkcol"][:])
        nc.sync.dma_start(epsc[:], aps["epsc"][:])
        for i in range(NB):
            nc.sync.dma_start(vaug[i][:, 128:129], aps["onesbc"][:])
        nc.vector.memset(kvf[:].bitcast(mybir.dt.uint32), 0)

        # pre-create all J's hst tiles so each J+1's first tiles (whose pool
        # buffers are free early) can be DMA'd ahead of the J boundary
        hst_all = [hst0] + [[hst_p.tile([128, JW], F32R, name="hst", tag="hst")
                             for _ in range(ND)] for _ in range(NJ - 1)]
        for d in range(6):
            nc.sync.dma_start(hst_all[1][d][:],
                              hsT[d * 128:(d + 1) * 128, JW:2 * JW])

        ones_b, hcol = cbt[:, 0:1], cbt[:, 1:2]

        def rotary(ps, dst):
            swp = rot_p.tile([128, JW], F32, name="rswp", tag="rswp", bufs=2)
            nc.vector.tensor_copy(swp[0:64, :], ps[64:128, :])
            nc.vector.tensor_copy(swp[64:128, :], ps[0:64, :])
            tmp = rot_p.tile([128, JW], F32, name="rtmp", tag="rtmp", bufs=2)
            nc.vector.tensor_mul(tmp[:], swp[:], si[:])
            nc.vector.tensor_mul(swp[:], ps[:], co[:])
            nc.vector.tensor_add(dst, swp[:], tmp[:])

        def emit_trans(Jp, pk_toks, pq_toks):
            # transposes + kv outer products for block Jp (exps long done)
            for t in range(4):
                c = 4 * Jp + t
                trk = ps1.tile([128, 128], BF16, name="trk", tag="work", bufs=4)
                nc.tensor.transpose(trk[:], pk_toks[t][:], identb[:])
                nc.vector.tensor_copy(pkT[c][:], trk[:])
                for h in range(2):
                    trq = ps1.tile([128, 128], BF16, name="trq", tag="work", bufs=4)
                    nc.tensor.transpose(trq[:], pq_toks[t][h][:], identb[:])
                    nc.vector.tensor_copy(pqT[c][h][:], trq[:])
                kvc = ps1.tile([128, 129], F32, name="kvc", tag="work", bufs=4)
                nc.tensor.matmul(kvc[:], pk_toks[t][:], vaug[c][:],
                                 start=True, stop=True)
                if c > 0:
                    nc.vector.tensor_copy(kvb[c - 1][:], kvf[:])
                nc.vector.tensor_add(kvf[:], kvf[:], kvc[:])

        prev_f = None
        for J in range(NJ):
            s0 = J * JW
            if J == 0:
                hst, co, si = hst0, co0, si0
            else:
                hst = hst_all[J]
                for d in range(6, ND):
                    nc.sync.dma_start(hst[d][:], hsT[d * 128:(d + 1) * 128, s0:s0 + JW])
                if J + 1 < NJ:
                    for d in range(6):
                        nc.sync.dma_start(hst_all[J + 1][d][:],
                                          hsT[d * 128:(d + 1) * 128,
                                              s0 + JW:s0 + 2 * JW])
                co = rot_p.tile([128, JW], F32, name="cos", tag="cos")
                si = rot_p.tile([128, JW], F32, name="sin", tag="sin")
                nc.sync.dma_start(co[:], aps["cost"][:, s0:s0 + JW])
                nc.sync.dma_start(si[:], aps["sintn"][:, s0:s0 + JW])

            # --- projections ---
            qtp = [sm1.tile([128, JW], BF16, name=f"qtp{g}", tag=f"qtp{g}", bufs=1)
                   for g in range(2)]
            for g in range(4):
                ps = ps1.tile([128, JW], F32, name="pp", tag="pp", bufs=2)
                for d in range(ND):
                    nc.tensor.matmul(ps[:], wq_t[d][:, g * 128:(g + 1) * 128],
                                     hst[d][:], start=(d == 0), stop=(d == ND - 1))
                if g < 2:
                    rotary(ps, qtp[g][:])
                else:
                    rotary(ps, qts[g - 2][:, s0:s0 + JW])
            # J-1's transposes/kv: PE work whose scalar-exp inputs completed
            # during this block's q matmuls
            if prev_f is not None:
                emit_trans(J - 1, *prev_f)
            ktp = sm1.tile([128, JW], BF16, name="ktp", tag="ktp", bufs=1)
            for g in range(2):
                ps = ps1.tile([128, JW], F32, name="pp", tag="pp", bufs=2)
                for d in range(ND):
                    nc.tensor.matmul(ps[:], wk_t[d][:, g * 128:(g + 1) * 128],
                                     hst[d][:], start=(d == 0), stop=(d == ND - 1))
                if g == 0:
                    rotary(ps, ktp[:])
                else:
                    rotary(ps, kts[:, s0:s0 + JW])
            # --- v projections + performer features, interleaved so the PE
            # always has independent matmul work while the vector/scalar
            # engines run the rotary and exp chains.
            def v_proj(sb):
                blk = J * 4 + sb
                ps = ps1.tile([128, 256], F32, name="ppv", tag="pp", bufs=2)
                for d in range(ND):
                    nc.tensor.matmul(ps[:], hst[d][:, sb * 128:(sb + 1) * 128],
                                     wv_t[d][:], start=(d == 0), stop=(d == ND - 1))
                nc.vector.tensor_copy(vaug[blk][:, 0:128], ps[:, 0:128])
                nc.scalar.copy(vs_t[blk][:], ps[:, 128:256])

            v_proj(0)
            v_proj(1)
            v_proj(2)
            v_proj(3)
            # q features (need only rotQ of heads 0/1 — done during k/v work)
            pq_toks = []
            for t in range(4):
                hp = []
                for h in range(2):
                    cs = t * 128
                    q2 = sm1.tile([128, 128], BF16, name="q2", tag="q2", bufs=2)
                    nc.vector.tensor_mul(q2[:], qtp[h][:, cs:cs + 128],
                                         qtp[h][:, cs:cs + 128])
                    fq = ps1.tile([128, 129], F32, name="fq", tag="work", bufs=4)
                    nc.tensor.matmul(fq[:, 0:128], qtp[h][:, cs:cs + 128], omgx[:],
                                     start=True, stop=True)
                    nc.tensor.matmul(fq[:, 128:129], q2[:], hcol, start=True, stop=True)
                    mx = sm1.tile([128, 1], F32, name="mx", tag="mx", bufs=4)
                    nc.vector.reduce_max(mx[:], fq[:, 0:128], axis=AX.X)
                    nc.vector.tensor_add(mx[:], mx[:], fq[:, 128:129])
                    nc.vector.tensor_scalar(mx[:], mx[:], -1.0, -LNM,
                                            ALU.mult, ALU.add)
                    pq_tok = sm1.tile([128, 128], BF16, name="pq", tag="pq", bufs=10)
                    nc.scalar.activation(pq_tok[:], fq[:, 0:128], AF.Exp,
                                         bias=mx[:], scale=1.0)
                    hp.append(pq_tok)
                pq_toks.append(hp)
            # k features (need rotK — done during the v work above)
            pk_toks = []
            for t in range(4):
                cs = t * 128
                k2 = sm1.tile([128, 128], BF16, name="k2", tag="k2", bufs=2)
                nc.vector.tensor_mul(k2[:], ktp[:, cs:cs + 128], ktp[:, cs:cs + 128])
                fk = ps1.tile([128, 129], F32, name="fk", tag="work", bufs=4)
                nc.tensor.matmul(fk[:, 0:128], ktp[:, cs:cs + 128], omgx[:],
                                 start=True, stop=True)
                nc.tensor.matmul(fk[:, 128:129], k2[:], hcol, start=True, stop=True)
                bk = sm1.tile([128, 1], F32, name="bk", tag="bk", bufs=4)
                nc.vector.tensor_scalar(bk[:], fk[:, 128:129], -1.0, stkcol[:],
                                        ALU.mult, ALU.add)
                pk_tok = sm1.tile([128, 128], BF16, name="pk", tag="pk", bufs=6)
                nc.scalar.activation(pk_tok[:], fk[:, 0:128], AF.Exp,
                                     bias=bk[:], scale=1.0)
                pk_toks.append(pk_tok)
            prev_f = (pk_toks, pq_toks)
            if J == 1:
                # pass-2-only tensors: issued late so they never delay the
                # compute-critical activation stream
                for t in range(4):
                    nc.sync.dma_start(dmask[t][:],
                                      aps["masks"][t * 128:(t + 1) * 128, :])
                for i in range(4):
                    nc.sync.dma_start(wo_t[i][:], wo[i * 128:(i + 1) * 128, :])
        emit_trans(NJ - 1, *prev_f)

    # ---------------- pass 2 ----------------
    with tc.tile_pool(name="pt2", bufs=3) as pt_p, \
         tc.tile_pool(name="sm2", bufs=3) as sm2, \
         tc.tile_pool(name="at2", bufs=2) as at_p, \
         tc.tile_pool(name="ost2", bufs=3) as ost_p, \
         tc.tile_pool(name="ps2", bufs=1, space="PSUM") as ps2:

        def emit_outproj(Jp, atiles):
            s0p = Jp * JW
            for sb in range(4):
                ostb = ost_p.tile([128, D], F32, name="ostb", tag="ost", bufs=3)
                for oc in range(4):
                    pso = ps2.tile([128, JW], F32, name="pso", tag="po", bufs=2)
                    for i in range(4):
                        nc.tensor.matmul(pso[:],
                                         atiles[i][:, sb * 128:(sb + 1) * 128],
                                         wo_t[i][:, oc * 512:(oc + 1) * 512],
                                         start=(i == 0), stop=(i == 3))
                    if oc % 2 == 0:
                        nc.vector.tensor_copy(ostb[:, oc * 512:(oc + 1) * 512],
                                              pso[:])
                    else:
                        nc.scalar.copy(ostb[:, oc * 512:(oc + 1) * 512], pso[:])
                nc.sync.dma_start(
                    out[s0p + sb * 128: s0p + (sb + 1) * 128, :], ostb[:])

        prev_at = None
        for J in range(NJ):
            s0 = J * JW
            nblk = 4 * J + 4
            # --- softmax heads ---
            av2 = [ps2.tile([128, JW], F32, name=f"av{h}", tag=f"av{h}", bufs=1)
                   for h in range(2)]
            dn2 = [ps2.tile([1, JW], F32, name=f"dn{h}", tag=f"dn{h}", bufs=1)
                   for h in range(2)]
            # software-pipelined: issue st(i) for both heads, then consume
            # pt(i-1) — the scalar exp always has a full block-time to finish
            # before the PE needs its output. Diagonal blocks (i >= 4J) only
            # compute the causally-needed column range [t*128, 512).
            pts = [None, None]
            pcs = 0
            for i in range(nblk + 1):
                npt = [None, None]
                ncs = (i - 4 * J) * 128 if i >= 4 * J else 0
                if i < nblk:
                    for h in range(2):
                        st = ps2.tile([128, JW], F32, name="st", tag="pp", bufs=2)
                        nc.tensor.matmul(st[:, ncs:JW], kts[:, i * 128:(i + 1) * 128],
                                         qts[h][:, s0 + ncs:s0 + JW],
                                         start=True, stop=True)
                        pt = pt_p.tile([128, JW], F32R, name="pt", tag="pt", bufs=4)
                        nc.scalar.activation(pt[:, ncs:JW], st[:, ncs:JW], AF.Exp,
                                             bias=0.0, scale=SCALE)
                        if i >= 4 * J:
                            nc.vector.tensor_mul(pt[:, ncs:JW], pt[:, ncs:JW],
                                                 dmask[i - 4 * J][:, ncs:JW])
                        npt[h] = pt
                if i > 0:
                    for h in range(2):
                        nc.tensor.matmul(av2[h][:, pcs:JW], vs_t[i - 1][:],
                                         pts[h][:, pcs:JW],
                                         start=(i == 1), stop=(i == nblk))
                        nc.tensor.matmul(dn2[h][:, pcs:JW], onesc[:],
                                         pts[h][:, pcs:JW],
                                         start=(i == 1), stop=(i == nblk))
                pts = npt
                pcs = ncs
            at_s = [at_p.tile([128, JW], BF16, name=f"ats{h}", tag=f"ats{h}")
                    for h in range(2)]
            for h in range(2):
                dnr = sm2.tile([1, JW], F32, name="dnr", tag="dnr", bufs=2)
                nc.scalar.activation(dnr[:], dn2[h][:], AF.Ln, bias=0.0, scale=1.0)
                nc.scalar.activation(dnr[:], dnr[:], AF.Exp, bias=0.0, scale=-1.0)
                bcs = sm2.tile([128, JW], F32, name="bcs", tag="bcs", bufs=2)
                nc.gpsimd.partition_broadcast(bcs[:], dnr[:])
                nc.vector.tensor_mul(at_s[h][:], av2[h][:], bcs[:])

            # --- performer heads: out = num/(den+EPS), den batched per (J,h)
            # into one [1,512] PSUM row (reusing the dn tag) so the
            # reciprocal/broadcast chain runs once per head per block.
            at_pf = [at_p.tile([128, JW], BF16, name=f"atp{h}", tag=f"atp{h}")
                     for h in range(2)]
            denJ = [ps2.tile([1, JW], F32, name=f"denp{h}", tag=f"dn{h}", bufs=1)
                    for h in range(2)]
            numJ = [sm2.tile([128, JW], F32, name=f"numc{h}", tag="numc", bufs=2)
                    for h in range(2)]
            for t in range(4):
                c = 4 * J + t
                cs = t * 128
                # both heads' aT first, so aM (vector) is ready by the time
                # the intra matmuls need it; inter matmuls need only kvb.
                aMs = [None, None]
                for h in range(2):
                    aT = ps2.tile([128, 128], F32, name="aT", tag="pp", bufs=2)
                    nc.tensor.matmul(aT[:], pkT[c][:], pqT[c][h][:],
                                     start=True, stop=True)
                    aM = sm2.tile([128, 128], BF16, name="aM", tag="aM", bufs=4)
                    nc.vector.tensor_mul(aM[:], aT[:], triu[:])
                    aMs[h] = aM
                for h in range(2):
                    num = ps2.tile([128, 128], F32, name="num", tag="pp", bufs=2)
                    if c > 0:
                        nc.tensor.matmul(num[:], kvb[c - 1][:, 0:128], pqT[c][h][:],
                                         start=True, stop=False)
                        nc.tensor.matmul(denJ[h][:, cs:cs + 128],
                                         kvb[c - 1][:, 128:129], pqT[c][h][:],
                                         start=True, stop=False)
                    nc.tensor.matmul(num[:], vaug[c][:, 0:128], aMs[h][:],
                                     start=(c == 0), stop=True)
                    nc.tensor.matmul(denJ[h][:, cs:cs + 128],
                                     vaug[c][:, 128:129], aMs[h][:],
                                     start=(c == 0), stop=True)
                    nc.vector.tensor_copy(numJ[h][:, cs:cs + 128], num[:])
            for h in range(2):
                rcp = sm2.tile([1, JW], F32, name="rcp", tag="rcp", bufs=2)
                nc.scalar.activation(rcp[:], denJ[h][:], AF.Ln,
                                     bias=epsc[0:1, :], scale=1.0)
                nc.scalar.activation(rcp[:], rcp[:], AF.Exp, bias=0.0, scale=-1.0)
                bcp = sm2.tile([128, JW], F32, name="bcp", tag="bcp", bufs=2)
                nc.gpsimd.partition_broadcast(bcp[:], rcp[:])
                nc.vector.tensor_mul(at_pf[h][:], numJ[h][:], bcp[:])

            # --- output projection of the PREVIOUS block: fills the PE while
            # this block's at_s/at_pf vector chains complete ---
            if prev_at is not None:
                emit_outproj(J - 1, prev_at)
            prev_at = [at_pf[0], at_pf[1], at_s[0], at_s[1]]

            if debug:
                for h in range(2):
                    nc.sync.dma_start(aps["dbg_ats"][h * 128:(h + 1) * 128, s0:s0 + JW],
                                      at_s[h][:])
                    nc.sync.dma_start(aps["dbg_atp"][h * 128:(h + 1) * 128, s0:s0 + JW],
                                      at_pf[h][:])
        emit_outproj(NJ - 1, prev_at)
        if debug:
            nc.sync.dma_start(aps["dbg_kts"][:], kts[:].bitcast(F32))
            for h in range(2):
                nc.sync.dma_start(aps["dbg_qts"][h * 128:(h + 1) * 128, :],
                                  qts[h][:].bitcast(F32))
            for c in range(NB):
                nc.sync.dma_start(aps["dbg_pk"][:, c * 128:(c + 1) * 128], pkT[c][:])
                for h in range(2):
                    nc.sync.dma_start(aps["dbg_pq"][h * 128:(h + 1) * 128,
                                                    c * 128:(c + 1) * 128], pqT[c][h][:])


def _pin_act_tables():
    """Make every ACT table-set except natural_log_exp_and_others ineligible so
    the loader never thrashes between table sets. Set ids are positional, so
    keep the dict size/order and just empty the others."""
    import concourse.bacc as bacc_mod
    if getattr(bacc_mod, "_act_tables_pinned", False):
        return
    orig = bacc_mod.get_activation_tables

    def patched(arch):
        t = orig(arch)
        return {k: (v if k == "natural_log_exp_and_others" else set())
                for k, v in t.items()}

    bacc_mod.get_activation_tables = patched
    bacc_mod._act_tables_pinned = True


def build(debug=False):
    _pin_act_tables()
    nc = bacc.Bacc("TRN2", target_bir_lowering=False, debug=False, num_devices=8)
    shapes = {
        "hsT": [D, S], "wq": [D, 512], "wk": [D, 256], "wv": [D, 256],
        "wo": [512, D], "cost": [128, S], "sintn": [128, S],
        "omgx": [128, 128], "identb": [128, 128], "triu": [128, 128],
        "cbt": [128, 2], "onesc": [128, 1], "stkcol": [128, 1],
        "epsc": [128, 1],
        "masks": [512, 512], "onesbc": [128, 1],
    }
    BF16_INS = {"omgx", "identb", "triu", "cbt", "masks", "onesbc"}
    F32R_INS = {"hsT", "wq", "wk", "wv", "onesc"}

    def _dt(n):
        if n == "wo":
            return BF16
        if n in BF16_INS:
            return BF16
        return F32R if n in F32R_INS else F32
    aps = {n: nc.dram_tensor(n, s, _dt(n), kind="ExternalInput").ap()
           for n, s in shapes.items()}
    aps["out"] = nc.dram_tensor("out", [S, D], F32, kind="ExternalOutput").ap()
    if debug:
        for n, s, dt in [("dbg_qts", [256, S], F32), ("dbg_kts", [128, S], F32),
                         ("dbg_ats", [256, S], BF16), ("dbg_atp", [256, S], BF16),
                         ("dbg_pq", [256, S], BF16), ("dbg_pk", [128, S], BF16)]:
            aps[n] = nc.dram_tensor(n, s, dt, kind="ExternalOutput").ap()
    with tile.TileContext(nc) as tc:
        _emit(tc, aps, debug=debug)
    nc.compile()
    return nc


def host_prep(hidden_states, cos, sin, Wq, Wk, Wv, Wo, omega):
    """Slice/transpose full inputs into 8 per-core input maps."""
    import ml_dtypes
    f32 = np.float32
    bf16 = ml_dtypes.bfloat16
    hs = np.asarray(hidden_states, f32)
    cos = np.asarray(cos, f32)
    sin = np.asarray(sin, f32)
    Wq, Wk, Wv, Wo = (np.asarray(x, f32) for x in (Wq, Wk, Wv, Wo))
    omega = np.asarray(omega, f32)

    omgx = np.ascontiguousarray((omega * HDQ).T).astype(bf16)
    identb = np.eye(128, dtype=f32).astype(bf16)
    triu = np.triu(np.ones((128, 128), f32)).astype(bf16)  # aT layout [k,q]: keep k<=q
    cbt = np.zeros((128, 2), f32)
    cbt[:, 0] = 1.0
    cbt[:, 1] = 0.5 * HD ** -0.5
    cbt = cbt.astype(bf16)
    onesc = np.ones((128, 1), f32)
    onesbc = np.ones((128, 1), f32).astype(bf16)
    masks = np.zeros((512, 512), f32)  # diag-block masks, 4x128
    pidx = np.arange(128)[:, None]
    cidx = np.arange(512)[None, :]
    for t in range(4):
        masks[t * 128:(t + 1) * 128, :] = (cidx >= t * 128 + pidx)
    masks = masks.astype(bf16)

    # stabk per (b, perf kv head j): max over (s,m) of proj_k (pre-stab)
    stab = np.zeros((B, 4), f32)
    kproj = np.einsum("bsd,od->bso", hs, Wk[0:512]).reshape(B, S, 4, HD)
    khalf = np.concatenate([-kproj[..., 64:], kproj[..., :64]], axis=-1)
    krot = kproj * cos[:, :, None, :] + khalf * sin[:, :, None, :]
    for b in range(B):
        for j in range(4):
            pj = (krot[b, :, j] * HDQ) @ omega.T
            stab[b, j] = pj.max()

    in_maps = []
    for core in range(8):
        b, j = divmod(core, 4)
        heads = [2 * j, 2 * j + 1, 8 + 2 * j, 8 + 2 * j + 1]
        qrows = np.concatenate([Wq[h * 128:(h + 1) * 128] for h in heads])
        kvh = [j, 4 + j]
        krows = np.concatenate([Wk[g * 128:(g + 1) * 128] for g in kvh])
        vrows = np.concatenate([Wv[g * 128:(g + 1) * 128] for g in kvh])
        wocols = np.concatenate([Wo[:, h * 128:(h + 1) * 128] for h in heads],
                                axis=1)
        sh = sin[b, :, 0:64]
        sintn = np.ascontiguousarray(np.concatenate([-sh, sh], axis=1).T)
        stkcol = np.full((128, 1), -stab[b, j] - LNM, f32)
        in_maps.append({
            "hsT": np.ascontiguousarray(hs[b].T),
            "wq": np.ascontiguousarray(qrows.T),
            "wk": np.ascontiguousarray(krows.T),
            "wv": np.ascontiguousarray(vrows.T),
            "wo": np.ascontiguousarray(wocols.T).astype(bf16),
            "cost": np.ascontiguousarray(cos[b].T),
            "sintn": sintn,
            "omgx": omgx, "identb": identb, "triu": triu,
            "cbt": cbt, "onesc": onesc, "stkcol": stkcol,
            "epsc": np.full((128, 1), EPS, f32),
            "masks": masks, "onesbc": onesbc,
        })
    return in_maps


_NC_CACHE = {}


def kernel(**inputs):
    from concourse.bass_utils import run_bass_kernel_spmd
    if "nc" not in _NC_CACHE:
        _NC_CACHE["nc"] = build(debug=False)
    nc = _NC_CACHE["nc"]
    in_maps = host_prep(**inputs)
    res = run_bass_kernel_spmd(nc, in_maps, core_ids=list(range(8)))
    out = np.zeros((B, S, D), np.float32)
    for core in range(8):
        out[core // 4] += res.results[core]["out"]
    return out


# revision 51
# speedup vs baseline: 3.2759x; 1.0553x over previous
"""Trainium2 Bass kernel for MixedPerformerAttention (B=2,S=2048,D=2048,H=16).

Sharding: 8 cores = 2 batches x 4 head-slots. Core c (b=c//4, j=c%4) owns
performer heads {2j, 2j+1} (kv head j) and softmax heads {8+2j, 8+2j+1}
(kv head 4+j), plus the matching Wq/Wk/Wv rows and Wo columns. Each core
computes a [S, D] partial output projection; the host sums 4 partials/batch.

Two-pass structure keeps the tensor engine continuously busy (PE ramps to
max clock only after ~3us of uninterrupted work):
  pass 1: q/k/v projections + rotary + performer FAVOR+ features
          (pq/pk in both layouts) + per-chunk kv outer products + prefix sums.
  pass 2: softmax attention (scores/exp/AV/denominator), performer causal
          linear attention (all chunk matmuls dependency-free thanks to the
          precomputed exclusive-prefix kv tensors), output projection.

dtypes: fp32r for every matmul with free-size >= 256 (full PE rate there),
bf16 only in the performer branch (free=128 matmuls where fp32r is 4x slower)
and for Wo/attn in the output projection. The exact reference stabilizers
(per-token q max, host-shipped global k max, sq, 1/sqrt(M)) are reproduced
so num/(den+EPS) matches the reference's EPS=1e-6 guard.
"""

import sys

sys.path.insert(0, "/opt/trn_rl_repo")

import numpy as np

import concourse.bass as bass
import concourse.tile as tile
from concourse import bacc, bass_isa, mybir
from concourse._compat import with_exitstack

F32 = mybir.dt.float32
F32R = mybir.dt.float32r
BF16 = mybir.dt.bfloat16
AF = mybir.ActivationFunctionType
AX = mybir.AxisListType
ALU = mybir.AluOpType

B, S, D = 2, 2048, 2048
H, KVH, HD = 16, 8, 128
NPH, M, C = 8, 128, 128
SCALE = HD ** -0.5
EPS = 1e-6
LNM = float(np.log(np.sqrt(M)))
HDQ = HD ** -0.25

NJ, JW, NB, ND = 4, 512, 16, 16


def _r(ap):
    return ap.bitcast(F32R)


@with_exitstack
def _emit(ctx, tc, aps, debug=False):
    nc = tc.nc
    hsT, wq, wk, wv, wo = aps["hsT"], aps["wq"], aps["wk"], aps["wv"], aps["wo"]
    out = aps["out"]

    pers = ctx.enter_context(tc.tile_pool(name="pers", bufs=1))

    # ---------------- persistent tiles ----------------
    omgx = pers.tile([128, 128], BF16, name="omgx", tag="omgx")
    identb = pers.tile([128, 128], BF16, name="identb", tag="identb")
    triu = pers.tile([128, 128], BF16, name="triu", tag="triu")
    cbt = pers.tile([128, 2], BF16, name="cbt", tag="cbt")  # col0 ones, col1 .5*HD^-.5
    onesc = pers.tile([128, 1], F32R, name="onesc", tag="onesc")
    stkcol = pers.tile([128, 1], F32, name="stkcol", tag="stkcol")  # -stabk - LNM
    epsc = pers.tile([128, 1], F32, name="epsc", tag="epsc")
    dmask = [pers.tile([128, 512], BF16, name=f"dmask{t}", tag=f"dmask{t}")
             for t in range(4)]
    wo_t = [pers.tile([128, 2048], BF16, name=f"wo{i}", tag=f"wo{i}") for i in range(4)]

    qts = [pers.tile([128, 2048], F32R, name=f"qts{h}", tag=f"qts{h}") for h in range(2)]
    kts = pers.tile([128, 2048], F32R, name="kts", tag="kts")
    vs_t = [pers.tile([128, 128], F32R, name=f"vs{i}", tag=f"vs{i}") for i in range(NB)]
    vaug = [pers.tile([128, 129], BF16, name=f"vaug{i}", tag=f"vaug{i}") for i in range(NB)]
    pqT = [[pers.tile([128, 128], BF16, name=f"pqT{i}_{h}", tag=f"pqT{i}_{h}")
            for h in range(2)] for i in range(NB)]
    pkT = [pers.tile([128, 128], BF16, name=f"pkT{i}", tag=f"pkT{i}") for i in range(NB)]
    kvb = [pers.tile([128, 129], BF16, name=f"kvb{i}", tag=f"kvb{i}") for i in range(1, NB)]
    kvf = pers.tile([128, 129], F32, name="kvf", tag="kvf")

    # ---------------- pass 1 ----------------
    with tc.tile_pool(name="w1", bufs=1) as w1, \
         tc.tile_pool(name="hstp", bufs=22) as hst_p, \
         tc.tile_pool(name="rot", bufs=2) as rot_p, \
         tc.tile_pool(name="sm1", bufs=3) as sm1, \
         tc.tile_pool(name="ps1", bufs=1, space="PSUM") as ps1:

        # d-blocked weight layouts: one big tile each, few big DMA descriptors
        # (DMA descriptors issue serially at ~0.7us each on the sync queue)
        wqb = w1.tile([128, ND * 512], F32R, name="wqb", tag="wqb")
        wkb = w1.tile([128, ND * 256], F32R, name="wkb", tag="wkb")
        wvb = w1.tile([128, ND * 256], F32R, name="wvb", tag="wvb")
        wq_t = [wqb[:, d * 512:(d + 1) * 512] for d in range(ND)]
        wk_t = [wkb[:, d * 256:(d + 1) * 256] for d in range(ND)]
        wv_t = [wvb[:, d * 256:(d + 1) * 256] for d in range(ND)]

        # hst: 4 d-blocks of [128, 2048] per J (d-blocked host layout "hsT"
        # = [128, NJ*4*2048]); hv(J, d) slices the right 512 columns.
        hstb = [[hst_p.tile([128, 2048], F32R, name="hstb", tag="hst", bufs=5)
                 for _ in range(4)] for _ in range(NJ)]

        def hv(J, d):
            return hstb[J][d // 4][:, (d % 4) * 512:(d % 4 + 1) * 512]

        def dma_hst(J, db):
            nc.sync.dma_start(hstb[J][db][:],
                              hsT[:, J * 8192 + db * 2048:J * 8192 + (db + 1) * 2048])

        # compute-critical DMAs first: J0 activations interleaved with q weights
        dma_hst(0, 0)
        nc.sync.dma_start(wqb[:, 0:4096], wq[:, 0:4096])
        dma_hst(0, 1)
        nc.sync.dma_start(wqb[:, 4096:8192], wq[:, 4096:8192])
        dma_hst(0, 2)
        co0 = rot_p.tile([128, JW], F32, name="cos", tag="cos")
        si0 = rot_p.tile([128, JW], F32, name="sin", tag="sin")
        nc.sync.dma_start(co0[:], aps["cost"][:, 0:JW])
        nc.sync.dma_start(si0[:], aps["sintn"][:, 0:JW])
        dma_hst(0, 3)
        nc.sync.dma_start(wkb[:], wk[:])
        nc.sync.dma_start(wvb[:], wv[:])
        dma_hst(1, 0)
        nc.sync.dma_start(omgx[:], aps["omgx"][:])
        nc.sync.dma_start(identb[:], aps["identb"][:])
        nc.sync.dma_start(triu[:], aps["triu"][:])
        nc.sync.dma_start(cbt[:], aps["cbt"][:])
        nc.sync.dma_start(onesc[:], aps["onesc"][:])
        nc.sync.dma_start(stkcol[:], aps["stkcol"][:])
        nc.sync.dma_start(epsc[:], aps["epsc"][:])
        for i in range(NB):
            nc.sync.dma_start(vaug[i][:, 128:129], aps["onesbc"][:])
        nc.vector.memset(kvf[:].bitcast(mybir.dt.uint32), 0)
        dma_hst(1, 1)

        ones_b, hcol = cbt[:, 0:1], cbt[:, 1:2]

        def rotary(ps, dst):
            swp = rot_p.tile([128, JW], F32, name="rswp", tag="rswp", bufs=2)
            nc.vector.tensor_copy(swp[0:64, :], ps[64:128, :])
            nc.vector.tensor_copy(swp[64:128, :], ps[0:64, :])
            tmp = rot_p.tile([128, JW], F32, name="rtmp", tag="rtmp", bufs=2)
            nc.vector.tensor_mul(tmp[:], swp[:], si[:])
            nc.vector.tensor_mul(swp[:], ps[:], co[:])
            nc.vector.tensor_add(dst, swp[:], tmp[:])

        def emit_trans(Jp, pk_toks, pq_toks):
            # transposes + kv outer products for block Jp (exps long done)
            for t in range(4):
                c = 4 * Jp + t
                trk = ps1.tile([128, 128], BF16, name="trk", tag="work", bufs=4)
                nc.tensor.transpose(trk[:], pk_toks[t][:], identb[:])
                nc.vector.tensor_copy(pkT[c][:], trk[:])
                for h in range(2):
                    trq = ps1.tile([128, 128], BF16, name="trq", tag="work", bufs=4)
                    nc.tensor.transpose(trq[:], pq_toks[t][h][:], identb[:])
                    nc.vector.tensor_copy(pqT[c][h][:], trq[:])
                kvc = ps1.tile([128, 129], F32, name="kvc", tag="work", bufs=4)
                nc.tensor.matmul(kvc[:], pk_toks[t][:], vaug[c][:],
                                 start=True, stop=True)
                if c > 0:
                    nc.vector.tensor_copy(kvb[c - 1][:], kvf[:])
                nc.vector.tensor_add(kvf[:], kvf[:], kvc[:])

        prev_f = None
        for J in range(NJ):
            s0 = J * JW
            if J == 0:
                co, si = co0, si0
            else:
                # co/si first — later hst descriptors can block the queue head
                co = rot_p.tile([128, JW], F32, name="cos", tag="cos")
                si = rot_p.tile([128, JW], F32, name="sin", tag="sin")
                nc.sync.dma_start(co[:], aps["cost"][:, s0:s0 + JW])
                nc.sync.dma_start(si[:], aps["sintn"][:, s0:s0 + JW])
                for db in range(2, 4):
                    dma_hst(J, db)
                if J + 1 < NJ:
                    dma_hst(J + 1, 0)
                    dma_hst(J + 1, 1)
            hst = [hv(J, d) for d in range(ND)]

            # --- projections ---
            qtp = [sm1.tile([128, JW], BF16, name=f"qtp{g}", tag=f"qtp{g}", bufs=1)
                   for g in range(2)]
            for g in range(4):
                ps = ps1.tile([128, JW], F32, name="pp", tag="pp", bufs=2)
                for d in range(ND):
                    nc.tensor.matmul(ps[:], wq_t[d][:, g * 128:(g + 1) * 128],
                                     hst[d][:], start=(d == 0), stop=(d == ND - 1))
                if g < 2:
                    rotary(ps, qtp[g][:])
                else:
                    rotary(ps, qts[g - 2][:, s0:s0 + JW])
            # J-1's transposes/kv: PE work whose scalar-exp inputs completed
            # during this block's q matmuls
            if prev_f is not None:
                emit_trans(J - 1, *prev_f)
            ktp = sm1.tile([128, JW], BF16, name="ktp", tag="ktp", bufs=1)
            for g in range(2):
                ps = ps1.tile([128, JW], F32, name="pp", tag="pp", bufs=2)
                for d in range(ND):
                    nc.tensor.matmul(ps[:], wk_t[d][:, g * 128:(g + 1) * 128],
                                     hst[d][:], start=(d == 0), stop=(d == ND - 1))
                if g == 0:
                    rotary(ps, ktp[:])
                else:
                    rotary(ps, kts[:, s0:s0 + JW])
            # --- v projections + performer features, interleaved so the PE
            # always has independent matmul work while the vector/scalar
            # engines run the rotary and exp chains.
            def v_proj(sb):
                blk = J * 4 + sb
                ps = ps1.tile([128, 256], F32, name="ppv", tag="pp", bufs=2)
                for d in range(ND):
                    nc.tensor.matmul(ps[:], hst[d][:, sb * 128:(sb + 1) * 128],
                                     wv_t[d][:], start=(d == 0), stop=(d == ND - 1))
                nc.vector.tensor_copy(vaug[blk][:, 0:128], ps[:, 0:128])
                nc.scalar.copy(vs_t[blk][:], ps[:, 128:256])

            v_proj(0)
            v_proj(1)
            v_proj(2)
            v_proj(3)
            # q features (need only rotQ of heads 0/1 — done during k/v work)
            pq_toks = []
            for t in range(4):
                hp = []
                for h in range(2):
                    cs = t * 128
                    q2 = sm1.tile([128, 128], BF16, name="q2", tag="q2", bufs=2)
                    nc.vector.tensor_mul(q2[:], qtp[h][:, cs:cs + 128],
                                         qtp[h][:, cs:cs + 128])
                    fq = ps1.tile([128, 129], F32, name="fq", tag="work", bufs=4)
                    nc.tensor.matmul(fq[:, 0:128], qtp[h][:, cs:cs + 128], omgx[:],
                                     start=True, stop=True)
                    nc.tensor.matmul(fq[:, 128:129], q2[:], hcol, start=True, stop=True)
                    mx = sm1.tile([128, 1], F32, name="mx", tag="mx", bufs=4)
                    nc.vector.reduce_max(mx[:], fq[:, 0:128], axis=AX.X)
                    nc.vector.tensor_add(mx[:], mx[:], fq[:, 128:129])
                    nc.vector.tensor_scalar(mx[:], mx[:], -1.0, -LNM,
                                            ALU.mult, ALU.add)
                    pq_tok = sm1.tile([128, 128], BF16, name="pq", tag="pq", bufs=10)
                    nc.scalar.activation(pq_tok[:], fq[:, 0:128], AF.Exp,
                                         bias=mx[:], scale=1.0)
                    hp.append(pq_tok)
                pq_toks.append(hp)
            # k features (need rotK — done during the v work above)
            pk_toks = []
            for t in range(4):
                cs = t * 128
                k2 = sm1.tile([128, 128], BF16, name="k2", tag="k2", bufs=2)
                nc.vector.tensor_mul(k2[:], ktp[:, cs:cs + 128], ktp[:, cs:cs + 128])
                fk = ps1.tile([128, 129], F32, name="fk", tag="work", bufs=4)
                nc.tensor.matmul(fk[:, 0:128], ktp[:, cs:cs + 128], omgx[:],
                                 start=True, stop=True)
                nc.tensor.matmul(fk[:, 128:129], k2[:], hcol, start=True, stop=True)
                bk = sm1.tile([128, 1], F32, name="bk", tag="bk", bufs=4)
                nc.vector.tensor_scalar(bk[:], fk[:, 128:129], -1.0, stkcol[:],
                                        ALU.mult, ALU.add)
                pk_tok = sm1.tile([128, 128], BF16, name="pk", tag="pk", bufs=6)
                nc.scalar.activation(pk_tok[:], fk[:, 0:128], AF.Exp,
                                     bias=bk[:], scale=1.0)
                pk_toks.append(pk_tok)
            prev_f = (pk_toks, pq_toks)
            if J == 1:
                # pass-2-only tensors: issued late so they never delay the
                # compute-critical activation stream
                for t in range(4):
                    nc.sync.dma_start(dmask[t][:],
                                      aps["masks"][t * 128:(t + 1) * 128, :])
                for i in range(4):
                    nc.sync.dma_start(wo_t[i][:], wo[i * 128:(i + 1) * 128, :])
        emit_trans(NJ - 1, *prev_f)

    # ---------------- pass 2 ----------------
    with tc.tile_pool(name="pt2", bufs=3) as pt_p, \
         tc.tile_pool(name="sm2", bufs=3) as sm2, \
         tc.tile_pool(name="at2", bufs=2) as at_p, \
         tc.tile_pool(name="ost2", bufs=3) as ost_p, \
         tc.tile_pool(name="ps2", bufs=1, space="PSUM") as ps2:

        def emit_outproj(Jp, atiles):
            s0p = Jp * JW
            for sb in range(4):
                ostb = ost_p.tile([128, D], F32, name="ostb", tag="ost", bufs=3)
                for oc in range(4):
                    pso = ps2.tile([128, JW], F32, name="pso", tag="po", bufs=2)
                    for i in range(4):
                        nc.tensor.matmul(pso[:],
                                         atiles[i][:, sb * 128:(sb + 1) * 128],
                                         wo_t[i][:, oc * 512:(oc + 1) * 512],
                                         start=(i == 0), stop=(i == 3))
                    if oc % 2 == 0:
                        nc.vector.tensor_copy(ostb[:, oc * 512:(oc + 1) * 512],
                                              pso[:])
                    else:
                        nc.scalar.copy(ostb[:, oc * 512:(oc + 1) * 512], pso[:])
                nc.sync.dma_start(
                    out[s0p + sb * 128: s0p + (sb + 1) * 128, :], ostb[:])

        prev_at = None
        for J in range(NJ):
            s0 = J * JW
            nblk = 4 * J + 4
            # --- softmax heads ---
            av2 = [ps2.tile([128, JW], F32, name=f"av{h}", tag=f"av{h}", bufs=1)
                   for h in range(2)]
            dn2 = [ps2.tile([1, JW], F32, name=f"dn{h}", tag=f"dn{h}", bufs=1)
                   for h in range(2)]
            # software-pipelined: issue st(i) for both heads, then consume
            # pt(i-1) — the scalar exp always has a full block-time to finish
            # before the PE needs its output. Diagonal blocks (i >= 4J) only
            # compute the causally-needed column range [t*128, 512).
            pts = [None, None]
            pcs = 0
            for i in range(nblk + 1):
                npt = [None, None]
                ncs = (i - 4 * J) * 128 if i >= 4 * J else 0
                if i < nblk:
                    for h in range(2):
                        st = ps2.tile([128, JW], F32, name="st", tag="pp", bufs=2)
                        nc.tensor.matmul(st[:, ncs:JW], kts[:, i * 128:(i + 1) * 128],
                                         qts[h][:, s0 + ncs:s0 + JW],
                                         start=True, stop=True)
                        pt = pt_p.tile([128, JW], F32R, name="pt", tag="pt", bufs=4)
                        nc.scalar.activation(pt[:, ncs:JW], st[:, ncs:JW], AF.Exp,
                                             bias=0.0, scale=SCALE)
                        if i >= 4 * J:
                            nc.vector.tensor_mul(pt[:, ncs:JW], pt[:, ncs:JW],
                                                 dmask[i - 4 * J][:, ncs:JW])
                        npt[h] = pt
                if i > 0:
                    for h in range(2):
                        nc.tensor.matmul(av2[h][:, pcs:JW], vs_t[i - 1][:],
                                         pts[h][:, pcs:JW],
                                         start=(i == 1), stop=(i == nblk))
                        nc.tensor.matmul(dn2[h][:, pcs:JW], onesc[:],
                                         pts[h][:, pcs:JW],
                                         start=(i == 1), stop=(i == nblk))
                pts = npt
                pcs = ncs
            at_s = [at_p.tile([128, JW], BF16, name=f"ats{h}", tag=f"ats{h}")
                    for h in range(2)]
            for h in range(2):
                dnr = sm2.tile([1, JW], F32, name="dnr", tag="dnr", bufs=2)
                nc.scalar.activation(dnr[:], dn2[h][:], AF.Ln, bias=0.0, scale=1.0)
                nc.scalar.activation(dnr[:], dnr[:], AF.Exp, bias=0.0, scale=-1.0)
                bcs = sm2.tile([128, JW], F32, name="bcs", tag="bcs", bufs=2)
                nc.gpsimd.partition_broadcast(bcs[:], dnr[:])
                nc.vector.tensor_mul(at_s[h][:], av2[h][:], bcs[:])

            # --- performer heads: out = num/(den+EPS), den batched per (J,h)
            # into one [1,512] PSUM row (reusing the dn tag) so the
            # reciprocal/broadcast chain runs once per head per block.
            at_pf = [at_p.tile([128, JW], BF16, name=f"atp{h}", tag=f"atp{h}")
                     for h in range(2)]
            denJ = [ps2.tile([1, JW], F32, name=f"denp{h}", tag=f"dn{h}", bufs=1)
                    for h in range(2)]
            numJ = [sm2.tile([128, JW], F32, name=f"numc{h}", tag="numc", bufs=2)
                    for h in range(2)]
            for t in range(4):
                c = 4 * J + t
                cs = t * 128
                # both heads' aT first, so aM (vector) is ready by the time
                # the intra matmuls need it; inter matmuls need only kvb.
                aMs = [None, None]
                for h in range(2):
                    aT = ps2.tile([128, 128], F32, name="aT", tag="pp", bufs=2)
                    nc.tensor.matmul(aT[:], pkT[c][:], pqT[c][h][:],
                                     start=True, stop=True)
                    aM = sm2.tile([128, 128], BF16, name="aM", tag="aM", bufs=4)
                    nc.vector.tensor_mul(aM[:], aT[:], triu[:])
                    aMs[h] = aM
                for h in range(2):
                    num = ps2.tile([128, 128], F32, name="num", tag="pp", bufs=2)
                    if c > 0:
                        nc.tensor.matmul(num[:], kvb[c - 1][:, 0:128], pqT[c][h][:],
                                         start=True, stop=False)
                        nc.tensor.matmul(denJ[h][:, cs:cs + 128],
                                         kvb[c - 1][:, 128:129], pqT[c][h][:],
                                         start=True, stop=False)
                    nc.tensor.matmul(num[:], vaug[c][:, 0:128], aMs[h][:],
                                     start=(c == 0), stop=True)
                    nc.tensor.matmul(denJ[h][:, cs:cs + 128],
                                     vaug[c][:, 128:129], aMs[h][:],
                                     start=(c == 0), stop=True)
                    nc.vector.tensor_copy(numJ[h][:, cs:cs + 128], num[:])
            for h in range(2):
                rcp = sm2.tile([1, JW], F32, name="rcp", tag="rcp", bufs=2)
                nc.scalar.activation(rcp[:], denJ[h][:], AF.Ln,
                                     bias=epsc[0:1, :], scale=1.0)
                nc.scalar.activation(rcp[:], rcp[:], AF.Exp, bias=0.0, scale=-1.0)
                bcp = sm2.tile([128, JW], F32, name="bcp", tag="bcp", bufs=2)
                nc.gpsimd.partition_broadcast(bcp[:], rcp[:])
                nc.vector.tensor_mul(at_pf[h][:], numJ[h][:], bcp[:])

            # --- output projection of the PREVIOUS block: fills the PE while
            # this block's at_s/at_pf vector chains complete ---
            if prev_at is not None:
                emit_outproj(J - 1, prev_at)
            prev_at = [at_pf[0], at_pf[1], at_s[0], at_s[1]]

            if debug:
                for h in range(2):
                    nc.sync.dma_start(aps["dbg_ats"][h * 128:(h + 1) * 128, s0:s0 + JW],
                                      at_s[h][:])
                    nc.sync.dma_start(aps["dbg_atp"][h * 128:(h + 1) * 128, s0:s0 + JW],
                                      at_pf[h][:])
        emit_outproj(NJ - 1, prev_at)
        if debug:
            nc.sync.dma_start(aps["dbg_kts"][:], kts[:].bitcast(F32))
            for h in range(2):
                nc.sync.dma_start(aps["dbg_qts"][h * 128:(h + 1) * 128, :],
                                  qts[h][:].bitcast(F32))
            for c in range(NB):
                nc.sync.dma_start(aps["dbg_pk"][:, c * 128:(c + 1) * 128], pkT[c][:])
                for h in range(2):
                    nc.sync.dma_start(aps["dbg_pq"][h * 128:(h + 1) * 128,
                                                    c * 128:(c + 1) * 128], pqT[c][h][:])


def _pin_act_tables():
    """Make every ACT table-set except natural_log_exp_and_others ineligible so
    the loader never thrashes between table sets. Set ids are positional, so
    keep the dict size/order and just empty the others."""
    import concourse.bacc as bacc_mod
    if getattr(bacc_mod, "_act_tables_pinned", False):
        return
    orig = bacc_mod.get_activation_tables

    def patched(arch):
        t = orig(arch)
        return {k: (v if k == "natural_log_exp_and_others" else set())
                for k, v in t.items()}

    bacc_mod.get_activation_tables = patched
    bacc_mod._act_tables_pinned = True


def build(debug=False):
    _pin_act_tables()
    nc = bacc.Bacc("TRN2", target_bir_lowering=False, debug=False, num_devices=8)
    shapes = {
        "hsT": [128, NJ * 4 * 2048], "wq": [128, ND * 512],
        "wk": [128, ND * 256], "wv": [128, ND * 256],
        "wo": [512, D], "cost": [128, S], "sintn": [128, S],
        "omgx": [128, 128], "identb": [128, 128], "triu": [128, 128],
        "cbt": [128, 2], "onesc": [128, 1], "stkcol": [128, 1],
        "epsc": [128, 1],
        "masks": [512, 512], "onesbc": [128, 1],
    }
    BF16_INS = {"omgx", "identb", "triu", "cbt", "masks", "onesbc"}
    F32R_INS = {"hsT", "wq", "wk", "wv", "onesc"}

    def _dt(n):
        if n == "wo":
            return BF16
        if n in BF16_INS:
            return BF16
        return F32R if n in F32R_INS else F32
    aps = {n: nc.dram_tensor(n, s, _dt(n), kind="ExternalInput").ap()
           for n, s in shapes.items()}
    aps["out"] = nc.dram_tensor("out", [S, D], F32, kind="ExternalOutput").ap()
    if debug:
        for n, s, dt in [("dbg_qts", [256, S], F32), ("dbg_kts", [128, S], F32),
                         ("dbg_ats", [256, S], BF16), ("dbg_atp", [256, S], BF16),
                         ("dbg_pq", [256, S], BF16), ("dbg_pk", [128, S], BF16)]:
            aps[n] = nc.dram_tensor(n, s, dt, kind="ExternalOutput").ap()
    with tile.TileContext(nc) as tc:
        _emit(tc, aps, debug=debug)
    nc.compile()
    return nc


def host_prep(hidden_states, cos, sin, Wq, Wk, Wv, Wo, omega):
    """Slice/transpose full inputs into 8 per-core input maps."""
    import ml_dtypes
    f32 = np.float32
    bf16 = ml_dtypes.bfloat16
    hs = np.asarray(hidden_states, f32)
    cos = np.asarray(cos, f32)
    sin = np.asarray(sin, f32)
    Wq, Wk, Wv, Wo = (np.asarray(x, f32) for x in (Wq, Wk, Wv, Wo))
    omega = np.asarray(omega, f32)

    omgx = np.ascontiguousarray((omega * HDQ).T).astype(bf16)
    identb = np.eye(128, dtype=f32).astype(bf16)
    triu = np.triu(np.ones((128, 128), f32)).astype(bf16)  # aT layout [k,q]: keep k<=q
    cbt = np.zeros((128, 2), f32)
    cbt[:, 0] = 1.0
    cbt[:, 1] = 0.5 * HD ** -0.5
    cbt = cbt.astype(bf16)
    onesc = np.ones((128, 1), f32)
    onesbc = np.ones((128, 1), f32).astype(bf16)
    masks = np.zeros((512, 512), f32)  # diag-block masks, 4x128
    pidx = np.arange(128)[:, None]
    cidx = np.arange(512)[None, :]
    for t in range(4):
        masks[t * 128:(t + 1) * 128, :] = (cidx >= t * 128 + pidx)
    masks = masks.astype(bf16)

    # stabk per (b, perf kv head j): max over (s,m) of proj_k (pre-stab)
    stab = np.zeros((B, 4), f32)
    kproj = np.einsum("bsd,od->bso", hs, Wk[0:512]).reshape(B, S, 4, HD)
    khalf = np.concatenate([-kproj[..., 64:], kproj[..., :64]], axis=-1)
    krot = kproj * cos[:, :, None, :] + khalf * sin[:, :, None, :]
    for b in range(B):
        for j in range(4):
            pj = (krot[b, :, j] * HDQ) @ omega.T
            stab[b, j] = pj.max()

    in_maps = []
    for core in range(8):
        b, j = divmod(core, 4)
        heads = [2 * j, 2 * j + 1, 8 + 2 * j, 8 + 2 * j + 1]
        qrows = np.concatenate([Wq[h * 128:(h + 1) * 128] for h in heads])
        kvh = [j, 4 + j]
        krows = np.concatenate([Wk[g * 128:(g + 1) * 128] for g in kvh])
        vrows = np.concatenate([Wv[g * 128:(g + 1) * 128] for g in kvh])
        wocols = np.concatenate([Wo[:, h * 128:(h + 1) * 128] for h in heads],
                                axis=1)
        sh = sin[b, :, 0:64]
        sintn = np.ascontiguousarray(np.concatenate([-sh, sh], axis=1).T)
        stkcol = np.full((128, 1), -stab[b, j] - LNM, f32)
        # d-blocked layouts: x[p, d*w + c] = xT[d*128 + p, c]
        hsb = np.ascontiguousarray(
            hs[b].T.reshape(16, 128, 4, 512).transpose(1, 2, 0, 3)
            .reshape(128, 4 * 16 * 512))
        wqb = np.ascontiguousarray(
            qrows.T.reshape(16, 128, 512).transpose(1, 0, 2).reshape(128, 8192))
        wkb = np.ascontiguousarray(
            krows.T.reshape(16, 128, 256).transpose(1, 0, 2).reshape(128, 4096))
        wvb = np.ascontiguousarray(
            vrows.T.reshape(16, 128, 256).transpose(1, 0, 2).reshape(128, 4096))
        in_maps.append({
            "hsT": hsb,
            "wq": wqb,
            "wk": wkb,
            "wv": wvb,
            "wo": np.ascontiguousarray(wocols.T).astype(bf16),
            "cost": np.ascontiguousarray(cos[b].T),
            "sintn": sintn,
            "omgx": omgx, "identb": identb, "triu": triu,
            "cbt": cbt, "onesc": onesc, "stkcol": stkcol,
            "epsc": np.full((128, 1), EPS, f32),
            "masks": masks, "onesbc": onesbc,
        })
    return in_maps


_NC_CACHE = {}


def kernel(**inputs):
    from concourse.bass_utils import run_bass_kernel_spmd
    if "nc" not in _NC_CACHE:
        _NC_CACHE["nc"] = build(debug=False)
    nc = _NC_CACHE["nc"]
    in_maps = host_prep(**inputs)
    res = run_bass_kernel_spmd(nc, in_maps, core_ids=list(range(8)))
    out = np.zeros((B, S, D), np.float32)
    for core in range(8):
        out[core // 4] += res.results[core]["out"]
    return out
